# revision 1
# baseline (speedup 1.0000x reference)
"""Trainium2 Bass kernel for the custom quaternion Huber loss.

Contract: kernel(**inputs) takes FULL unsharded numpy inputs (keyed as in
setup_inputs) and returns the full scalar output. Internally the batch is
sharded data-parallel across 8 NeuronCores; the small quaternion table
gather and the batch_X time-slice are done host-side as part of sharding
(pure data movement + index arithmetic); all floating-point math of the
loss runs on-device.

Math notes (exact reformulations, no approximation beyond fp16/Taylor
truncation ~1e-8):
  - reference normalizes q0, rot, and diff; since diff is normalized last
    and atan2 / v/|v| are invariant under positive scaling, the q0 and rot
    normalizations cancel exactly.  We use the scaled rotation
        rot' = rot * |w| / sin(h) = [ |w|*cot(h), w ],  h = 0.5*DT*|w|
    and |w|*cot(h) = (2/DT)*(h*cot(h)) = B0 + B1*|w|^2 + O(h^4)  (Taylor,
    h <= ~0.04 for randn inputs, truncation < 2e-8 relative).
  - diff = qmul(conj(computed), tq); tq is pre-scaled by 1/256 to keep the
    fp16 dynamic range comfortable (scale cancels too).
  - angle = 2*atan2(|v|, w) = pi - 2*atan(w/|v|)   (|v| > 0)
  - huber(a) with delta=1 == m*(a - 0.5*m) where m = min(a, 1).
"""

import math
import os

import numpy as np

P = 128
NCORES = 8
DT = 0.01

_CACHE = {}


def _build_module(bs):
    """Build + compile the per-core Bass module for a per-core batch of bs."""
    import concourse.bacc as bacc
    import concourse.tile as tile
    from concourse import mybir

    fd = bs // P
    assert fd * P == bs
    f32 = mybir.dt.float32
    f16 = mybir.dt.float16
    OP = mybir.AluOpType
    AF = mybir.ActivationFunctionType

    B0 = 2.0 / DT                              # 200
    B1 = -(2.0 / DT) * (DT / 2.0) ** 2 / 3.0   # -200 * 2.5e-5 / 3
    TSCALE = 1.0 / 256.0

    nc = bacc.Bacc(
        "TRN2",
        target_bir_lowering=False,
        debug=False,
        enable_asserts=False,
        num_devices=NCORES,
    )

    an_d = nc.dram_tensor("an", (bs, 3), f32, kind="ExternalInput").ap()
    bi_d = nc.dram_tensor("bi", (bs, 3), f32, kind="ExternalInput").ap()
    q0_d = nc.dram_tensor("q0", (4, bs), f32, kind="ExternalInput").ap()
    tq_d = nc.dram_tensor("tq", (4, bs), f32, kind="ExternalInput").ap()
    acc_d = nc.dram_tensor("acc", (P, 2), f32, kind="ExternalOutput").ap()

    # qmul tables: per output component, 4 terms (sign, a_comp, b_comp).
    # out = qmul(a, b) per reference._qmul.
    QM = [
        [(+1, 0, 0), (-1, 1, 1), (-1, 2, 2), (-1, 3, 3)],
        [(+1, 0, 1), (+1, 1, 0), (+1, 2, 3), (-1, 3, 2)],
        [(+1, 0, 2), (-1, 1, 3), (+1, 2, 0), (+1, 3, 1)],
        [(+1, 0, 3), (+1, 1, 2), (-1, 2, 1), (+1, 3, 0)],
    ]
    # D = qmul(conj(C), T): flip sign of terms with a_comp in {1,2,3}
    QMC = [
        [(s if a == 0 else -s, a, b) for (s, a, b) in row] for row in QM
    ]

    with tile.TileContext(nc) as tc:
        with tc.tile_pool(name="main", bufs=1) as pool, tc.tile_pool(
            name="prod", bufs=2
        ) as prod:
            an_t = pool.tile([P, fd, 3], f32, tag="an_t")
            bi_t = pool.tile([P, fd, 3], f32, tag="bi_t")
            q0_t = pool.tile([P, 4, fd], f32, tag="q0_t")
            tq_t = pool.tile([P, 4, fd], f32, tag="tq_t")

            nc.sync.dma_start(out=an_t[:], in_=an_d.rearrange("(p f) c -> p f c", p=P))
            nc.sync.dma_start(out=bi_t[:], in_=bi_d.rearrange("(p f) c -> p f c", p=P))
            nc.sync.dma_start(out=q0_t[:], in_=q0_d.rearrange("c (p f) -> p c f", p=P))
            nc.sync.dma_start(out=tq_t[:], in_=tq_d.rearrange("c (p f) -> p c f", p=P))

            # ---- stage A: w = ang - bias (SoA fp16), s = |w|^2, rotw ----
            w3 = pool.tile([P, 3, fd], f16, tag="w3")
            nc.vector.tensor_sub(
                w3[:],
                an_t[:].rearrange("p f c -> p c f"),
                bi_t[:].rearrange("p f c -> p c f"),
            )
            sq3 = pool.tile([P, 3, fd], f16, tag="sq3")
            nc.scalar.activation(sq3[:], w3[:], AF.Square)
            s_a = pool.tile([P, fd], f16, tag="s_a")
            nc.vector.tensor_add(s_a[:], sq3[:, 0, :], sq3[:, 1, :])
            s_b = pool.tile([P, fd], f16, tag="s_b")
            nc.vector.tensor_add(s_b[:], s_a[:], sq3[:, 2, :])
            rotw = pool.tile([P, fd], f16, tag="rotw")
            nc.vector.tensor_scalar(rotw[:], s_b[:], B1, B0, OP.mult, OP.add)

            # ---- casts to fp16 SoA (tq pre-scaled by 1/256) ----
            q016 = pool.tile([P, 4, fd], f16, tag="q016")
            nc.scalar.activation(q016[:], q0_t[:], AF.Copy)
            tq16 = pool.tile([P, 4, fd], f16, tag="tq16")
            nc.scalar.activation(tq16[:], tq_t[:], AF.Copy, scale=TSCALE)

            def qmul_planes(out4, a_pl, b_pl, table):
                for c in range(4):
                    (s0, a0, b0), (s1, a1, b1), (s2, a2, b2), (s3, a3, b3) = table[c]
                    assert s0 == 1
                    ts = []
                    for k, (ai, bi_) in enumerate(
                        [(a0, b0), (a1, b1), (a2, b2), (a3, b3)]
                    ):
                        t = prod.tile([P, fd], f16, tag=f"t{k}")
                        nc.vector.tensor_mul(t[:], a_pl[ai], b_pl[bi_])
                        ts.append(t)
                    u0 = prod.tile([P, fd], f16, tag="u0")
                    nc.vector.tensor_tensor(
                        u0[:], ts[0][:], ts[1][:],
                        op=OP.add if s1 > 0 else OP.subtract,
                    )
                    u1 = prod.tile([P, fd], f16, tag="u1")
                    nc.vector.tensor_tensor(
                        u1[:], ts[2][:], ts[3][:],
                        op=OP.add if s2 * s3 > 0 else OP.subtract,
                    )
                    nc.vector.tensor_tensor(
                        out4[:, c, :], u0[:], u1[:],
                        op=OP.add if s2 > 0 else OP.subtract,
                    )

            # ---- stage B: C = qmul(q0, rot') ----
            C4 = pool.tile([P, 4, fd], f16, tag="C4")
            a_pl = [q016[:, j, :] for j in range(4)]
            r_pl = [rotw[:], w3[:, 0, :], w3[:, 1, :], w3[:, 2, :]]
            qmul_planes(C4, a_pl, r_pl, QM)

            # ---- stage C: D = qmul(conj(C), tq/256) ----
            D4 = pool.tile([P, 4, fd], f16, tag="D4")
            c_pl = [C4[:, j, :] for j in range(4)]
            t_pl = [tq16[:, j, :] for j in range(4)]
            qmul_planes(D4, c_pl, t_pl, QMC)

            # ---- stage D: angle/log-map/huber ----
            # angle = pi - 2*atan(q), q = Dw/|Dv| unbounded; ACT Arctan only
            # covers [-pi/2, pi/2], so range-reduce branchlessly:
            #   |q| <= 1 : atan(q) directly
            #   |q| >  1 : atan(q) = sign(q)*pi/2 - atan(1/q)
            CLIP = 1.57079
            # reuse an_t's slot for dsq (an/bi are dead after w3)
            dsq = pool.tile([P, 3, fd], f32, tag="an_t")
            nc.scalar.activation(dsq[:], D4[:, 1:4, :], AF.Square)
            v2a = pool.tile([P, fd], f32, tag="v2a")
            nc.vector.tensor_add(v2a[:], dsq[:, 0, :], dsq[:, 1, :])
            v2 = pool.tile([P, fd], f32, tag="v2")
            nc.vector.tensor_add(v2[:], v2a[:], dsq[:, 2, :])
            # guard against exact-zero |v| (would NaN through recip)
            nc.vector.tensor_scalar(v2[:], v2[:], 1e-30, None, OP.max)
            sv = pool.tile([P, fd], f32, tag="sv")
            nc.scalar.activation(sv[:], v2[:], AF.Sqrt)
            zs = pool.tile([P, fd], f32, tag="zs")
            nc.vector.reciprocal_approx_fast(zs[:], sv[:])
            u32 = pool.tile([P, fd], f32, tag="q0_t")  # q0_t input is dead
            nc.vector.tensor_copy(u32[:], D4[:, 0, :])
            q_r = pool.tile([P, fd], f32, tag="tq_t")  # tq_t input is dead
            nc.vector.tensor_mul(q_r[:], u32[:], zs[:])
            iu = pool.tile([P, fd], f32, tag="v2a")
            nc.vector.reciprocal_approx_fast(iu[:], u32[:])
            p_r = pool.tile([P, fd], f32, tag="v2")
            nc.vector.tensor_mul(p_r[:], sv[:], iu[:])
            qc = pool.tile([P, fd], f32, tag="qc")
            nc.vector.tensor_scalar(qc[:], q_r[:], CLIP, None, OP.min)
            nc.vector.tensor_scalar(qc[:], qc[:], -CLIP, None, OP.max)
            absq = pool.tile([P, fd], f32, tag="absq")
            nc.scalar.activation(absq[:], q_r[:], AF.Abs)
            mask = pool.tile([P, fd], mybir.dt.int32, tag="s_b")
            nc.vector.tensor_scalar(mask[:], absq[:], 1.0, None, OP.is_le)
            sgn = pool.tile([P, fd], f32, tag="sgn")
            nc.scalar.activation(sgn[:], q_r[:], AF.Sign)
            pc = pool.tile([P, fd], f32, tag="pc")
            nc.vector.tensor_scalar(pc[:], p_r[:], CLIP, None, OP.min)
            nc.vector.tensor_scalar(pc[:], pc[:], -CLIP, None, OP.max)
            at_q = pool.tile([P, fd], f32, tag="at_q")
            nc.scalar.activation(at_q[:], qc[:], AF.Arctan)
            at_p = pool.tile([P, fd], f32, tag="at_p")
            nc.scalar.activation(at_p[:], pc[:], AF.Arctan)
            alt = pool.tile([P, fd], f32, tag="s_a")
            nc.vector.scalar_tensor_tensor(
                alt[:], sgn[:], math.pi / 2.0, at_p[:], OP.mult, OP.subtract
            )
            at_full = pool.tile([P, fd], f32, tag="rotw")  # rotw dead
            nc.vector.tensor_copy(at_full[:], alt[:])
            nc.vector.copy_predicated(at_full[:], mask[:], at_q[:])
            pa = pool.tile([P, fd], f32, tag="w3")  # w3 dead after qmul1
            nc.vector.tensor_scalar(pa[:], at_full[:], -2.0, math.pi, OP.mult, OP.add)
            g = pool.tile([P, fd], f32, tag="qc")
            nc.vector.tensor_mul(g[:], pa[:], zs[:])
            # absD reuses sq3's slot (dead after s_b)
            absD = pool.tile([P, 3, fd], f16, tag="sq3")
            nc.scalar.activation(absD[:], D4[:, 1:4, :], AF.Abs)
            aL = pool.tile([P, 3, fd], f16, tag="aL")
            for j in range(3):
                nc.vector.tensor_mul(aL[:, j, :], absD[:, j, :], g[:])
            # huber(a) = 0.5*a^2 - 0.5*relu(a-1)^2 -> two fused ACT reduces
            acc2 = pool.tile([P, 2], f32, tag="acc")
            junkA = pool.tile([P, 3, fd], f16, tag="bi_t")
            nc.scalar.activation(
                junkA[:], aL[:], AF.Square, accum_out=acc2[:, 0:1]
            )
            bneg1 = pool.tile([P, 1], f32, tag="bneg1")
            nc.vector.memset(bneg1[:], -1.0)
            rl = pool.tile([P, 3, fd], f16, tag="an_t")  # after dsq
            nc.scalar.activation(rl[:], aL[:], AF.Relu, bias=bneg1[:], scale=1.0)
            junkB = pool.tile([P, 3, fd], f16, tag="s13")
            nc.scalar.activation(
                junkB[:], rl[:], AF.Square, accum_out=acc2[:, 1:2]
            )
            nc.sync.dma_start(out=acc_d, in_=acc2[:])

    nc.compile()
    return nc


def _get_module(bs):
    if bs not in _CACHE:
        _CACHE[bs] = _build_module(bs)
    return _CACHE[bs]


def _host_prep(true_quaternions, predicted_biases, batch_X, quaternions_all,
               indices, sequence_length):
    """Shard the full inputs into per-core input maps (data movement only)."""
    tq = np.asarray(true_quaternions, dtype=np.float32)
    bi = np.asarray(predicted_biases, dtype=np.float32)
    bx = np.asarray(batch_X)
    table = np.asarray(quaternions_all, dtype=np.float32)
    idx = np.asarray(indices)

    B = tq.shape[0]
    bs = B // NCORES
    seq = int(sequence_length)

    an = np.ascontiguousarray(bx[:, -1, 3:6], dtype=np.float32)       # [B,3]
    init_idx = np.maximum(idx.astype(np.int64) - (seq - 1), 0)
    q0 = table[init_idx]                                              # [B,4]

    # SoA per core: [NCORES, 4, bs]
    tqT = np.ascontiguousarray(tq.reshape(NCORES, bs, 4).transpose(0, 2, 1))
    q0T = np.ascontiguousarray(q0.reshape(NCORES, bs, 4).transpose(0, 2, 1))

    in_maps = []
    for c in range(NCORES):
        in_maps.append({
            "an": an[c * bs:(c + 1) * bs],
            "bi": bi[c * bs:(c + 1) * bs],
            "q0": q0T[c],
            "tq": tqT[c],
        })
    return in_maps, B, bs


def _run_traced(nc, in_maps):
    """Run once warm, then capture an NTFF profile of a second run and
    report per-core HW exec time (max across cores)."""
    import ctypes
    import glob
    import tempfile

    import jax
    from concourse import bass2jax

    jax.devices()
    results = bass2jax.run_bass_via_pjrt(nc, in_maps, n_cores=NCORES)  # warm

    lib = ctypes.CDLL("/opt/axon/libaxon_pjrt.so")
    lib.axon_start_nrt_profile.argtypes = [
        ctypes.POINTER(ctypes.c_int64), ctypes.c_size_t,
    ]
    lib.axon_start_nrt_profile.restype = ctypes.c_int64
    lib.axon_stop_nrt_profile.argtypes = [ctypes.c_char_p]
    lib.axon_stop_nrt_profile.restype = ctypes.c_int64

    tmpdir = tempfile.mkdtemp(prefix="qk_ntff_")
    rc = lib.axon_start_nrt_profile(None, 0)
    if rc != 0:
        print(f"profile start failed rc={rc}")
        return results, None
    try:
        results = bass2jax.run_bass_via_pjrt(nc, in_maps, n_cores=NCORES)
    finally:
        n = lib.axon_stop_nrt_profile(tmpdir.encode())
        print(f"profile: {n} file(s) written to {tmpdir}")

    ntffs = glob.glob(os.path.join(tmpdir, "*.ntff"))
    if not ntffs:
        print("no ntffs captured")
        return results, None

    import gauge.profiler
    from concourse._compat import FishPath

    profile = gauge.profiler.Profile(
        profile_path=FishPath(tmpdir),
        kernel_dev_mode=True,
        profile_on_exit=False,
        bass_kernel=nc.m,
        offline_processing=True,
        fname="*_body*",
        metadata={},
    )
    idxs = tuple(range(NCORES))
    profile.convert_ntffs_to_json(idxs)
    times = []
    for i in sorted(profile._model_indices_with_json):
        try:
            times.append((i, profile.get_total_time(i)))
        except Exception:
            pass
    if not times:
        print("ntff->json produced no usable summaries")
        return results, None
    print("per-core total_time:", times)
    return results, max(t for _, t in times)


def kernel(true_quaternions, predicted_biases, batch_X, quaternions_all,
           indices, sequence_length):
    from concourse import bass_utils

    in_maps, B, bs = _host_prep(
        true_quaternions, predicted_biases, batch_X, quaternions_all,
        indices, sequence_length,
    )
    nc = _get_module(bs)

    trace = os.environ.get("QK_TRACE", "0") == "1"
    if trace:
        try:
            results, exec_ns = _run_traced(nc, in_maps)
            if exec_ns is not None:
                print(f"HW exec time: {exec_ns} ns")
        except Exception as e:
            print(f"trace failed ({e!r}); falling back to plain run")
            res = bass_utils.run_bass_kernel_spmd(
                nc, in_maps, core_ids=list(range(NCORES)), trace=False
            )
            results = res.results
    else:
        res = bass_utils.run_bass_kernel_spmd(
            nc, in_maps, core_ids=list(range(NCORES)), trace=False
        )
        results = res.results

    total = 0.0
    for r in results:
        a = r["acc"].astype(np.float64)
        total += 0.5 * (a[:, 0].sum() - a[:, 1].sum())
    return np.float32(total / (3.0 * B))

